# revision 89
# baseline (speedup 1.0000x reference)
"""Trainium2 Bass kernel for BinaryMaskPredictor (ragged anchors), rev 3.

Data-parallel over the 256 anchors: 32 anchors per NeuronCore on 8 cores.
Host pre-stages per-anchor crops contiguously (layout/gather prep only —
all conv/BCE math stays on device), so crop DMAs are 1KB-run linear
transfers instead of 32B-run gathers (57us -> 12us of DMA mutex time),
and no runtime addressing (values_load) is needed at all.  Crop chunks
load progressively (1,1,2,4,8,8,8 anchors) so PE starts after ~6us.

Structure (all matmuls fp8e4; DoubleRow = 0.5 cyc/row):
  * Crops staged at pitch 32, rows -2..33 zero-padded, 1184B stride.
    DR pairs need K-rows 64B apart = (dy+2,dx): pairs (0,6),(1,7),
    (2,8) + zero-second-tile singles 3,4,5: 6 DR matmuls per
    (hf, 512-px chunk), psum [128,512] ring 3.
  * Out cols 0/31 read across rows; clean values are recomputed into a
    side psum bank from the in-bounds pairs only, then overwrite the h
    edge columns during the relu/fp8 copy (strided views).
  * h dense [128, 2048] fp8; stage A (Z = W2^T h) is one DR matmul per
    512-chunk into a 1-bank [16,512] psum, pipelined one anchor late so
    relu has a full conv1 of slack; chunks ride after different hf
    halves so the single bank never stalls PE.
  * Z copied (scaled 1/2048, +b2 on center-tap row) into pitch-32 z
    tiles, 4 anchors at partition 32j; stage B = 6 DR matmuls per
    512-chunk with M=32 (per-subgroup e9 routes anchors to psum rows
    4sg+j, other rows += 0), accumulating all 8 subgroups into one
    [32, 1024] logits psum; logit edge cols likewise accumulate into
    their own [32, 64] bank (start=True zeroes a whole 2KB psum
    region, so once-started accumulators can't share a bank with the
    per-anchor-restarted conv1 side columns).
  * BCE once at the end on [32, 1024] tiles, reading the logits psum
    directly (interior cols strided, edge cols from edt — no sbuf
    materialization): one gpsimd is_equal (int32 seg crop vs f32
    class), Exp + Ln(1+e)-with-accum_out on ACT, mult + row-reduce on
    DVE.  Only 2 activation-table loads (vs 16 when Exp/Ln alternate).
  * Elementwise psum consumers alternate ACT (hf0 relu, even z-copies)
    and DVE (hf1 relu, odd z-copies); gpsimd handles the seg compare.
PSUM banks: conv1 3x[128,512] (3) + zps [16,512] (1) + logits
[128,1024] (2) + logit edges [32,64] (1) + conv1 side [128,128] (1) = 8.
"""

import numpy as np
from contextlib import ExitStack

C = 128
HF = WF = 320
IMG = 1280
NANCH = 256
CROP = 32
NCORES = 8
APC = NANCH // NCORES     # 32 anchors per core
GRP = 4                   # anchors per stage-B subgroup
NSG = APC // GRP          # 8 subgroups per core
W1S = 32.0                # fp8 scale for W1 (+b1)
W2S = 64.0                # fp8 scale for W2
ZSC = 1.0 / (W1S * W2S)   # undo scaling at the Z copy
XD = 64                   # staging offset of crop row 0 (2 pad rows)
ALEN = 36 * 32            # staged region: rows -2..33, pitch 32
XBYTES = ALEN + 32        # per-anchor stride (slack: zero-tile DR rows
                          # of singles read up to ALEN+0 inclusive)
# progressive crop-DMA chunking: small first chunks so PE starts early
DMACHUNKS = [1, 1, 2, 4, 8, 8, 8]
XLEN = 1160               # z tile pitch-32 domain (64 + 32*34 + slack)
# DoubleRow pairs (stride 64 = 2 rows x pitch 32): (tap, tap+6), plus
# zero-second-tile singles for taps 3,4,5
C1PAIRS = [(0, 6), (1, 7), (2, 8), (3, None), (4, None), (5, None)]
ZPAIRS = C1PAIRS

_cache = {}
last_exec_time_ns = None
last_results = None


def _build_program():
    import concourse.bass as bass
    import concourse.tile as tile
    import concourse.mybir as mybir
    from concourse import bacc

    f32 = mybir.dt.float32
    f8 = mybir.dt.float8e4
    i32 = mybir.dt.int32
    AF = mybir.ActivationFunctionType
    OP = mybir.AluOpType
    PM = mybir.MatmulPerfMode

    nc = bacc.Bacc("TRN2", target_bir_lowering=False, debug=False,
                   num_devices=NCORES)

    # anchor -> (chunk index, position in chunk)
    chunk_of = {}
    chunk_start = []
    s = 0
    for k, n in enumerate(DMACHUNKS):
        chunk_start.append(s)
        for i in range(n):
            chunk_of[s + i] = (k, i)
        s += n
    assert s == APC
    xs = nc.declare_dram_parameter("xs", [C, APC * XBYTES], f8,
                                   isOutput=False)
    msegp = nc.declare_dram_parameter("msegp", [APC, 1024], i32,
                                      isOutput=False)
    clsp = nc.declare_dram_parameter("clsp", [APC, 1], f32, isOutput=False)
    w1t = nc.declare_dram_parameter("w1t", [C, 3072], f8, isOutput=False)
    w2t = nc.declare_dram_parameter("w2t", [C, 32], f8, isOutput=False)
    e9t = nc.declare_dram_parameter("e9t", [C, 3072], f8, isOutput=False)
    b1t = nc.declare_dram_parameter("b1t", [C, 2], f32, isOutput=False)
    b2t = nc.declare_dram_parameter("b2t", [C, 1], f32, isOutput=False)
    outp = nc.declare_dram_parameter("out", [APC, 1], f32, isOutput=True)

    def view(t, off, dims, nparts=None):
        v = t[:]
        p0 = list(v.ap[0]) if nparts is None else [v.ap[0][0], nparts]
        return bass.AP(v.tensor, v.offset + off, [p0] + dims)

    with ExitStack() as ctx:
        tc = ctx.enter_context(tile.TileContext(nc))

        consts = ctx.enter_context(tc.tile_pool(name="consts", bufs=1))

        c1p = ctx.enter_context(tc.tile_pool(name="c1p", bufs=3, space="PSUM"))
        zpp = ctx.enter_context(tc.tile_pool(name="zpp", bufs=1, space="PSUM"))
        ltp = ctx.enter_context(tc.tile_pool(name="ltp", bufs=1, space="PSUM"))
        edp = ctx.enter_context(tc.tile_pool(name="edp", bufs=1, space="PSUM"))
        sdp = ctx.enter_context(tc.tile_pool(name="sdp", bufs=1, space="PSUM"))

        # ---- weights / constants / crop staging ----
        w1_sb = consts.tile([C, 3072], f8)
        b1_sb = consts.tile([C, 2], f32)
        w2_sb = consts.tile([C, 32], f8)
        e9_sb = consts.tile([C, 3072], f8)
        b2_sb = consts.tile([C, 1], f32)
        cls_sb = consts.tile([APC, 1], f32)
        mseg_sb = consts.tile([APC, 1024], i32)
        # crop staging tiles: one per DMA chunk so deps stay per-chunk
        x_tiles = [consts.tile([C, n * XBYTES], f8, name=f"xc{k}")
                   for k, n in enumerate(DMACHUNKS)]

        def crop_dma(k):
            o = chunk_start[k] * XBYTES
            nc.sync.dma_start(out=x_tiles[k][:],
                              in_=xs[:, o:o + DMACHUNKS[k] * XBYTES])

        # DMA order: only w1 + b1 + the first (1-anchor) crop chunk gate
        # the first conv1; everything else rides behind
        nc.sync.dma_start(out=w1_sb[:], in_=w1t[:])
        nc.sync.dma_start(out=b1_sb[:], in_=b1t[:])
        crop_dma(0)
        crop_dma(1)
        nc.scalar.dma_start(out=w2_sb[:], in_=w2t[:])
        crop_dma(2)
        nc.scalar.dma_start(out=b2_sb[:], in_=b2t[:])
        crop_dma(3)
        nc.scalar.dma_start(out=e9_sb[:], in_=e9t[:])
        nc.scalar.dma_start(out=cls_sb[:], in_=clsp[:])
        nc.scalar.dma_start(out=mseg_sb[:], in_=msegp[:])
        for k in range(4, len(DMACHUNKS)):
            crop_dma(k)

        z_tiles = [consts.tile([C, XLEN], f8, name=f"zt{i}")
                   for i in range(3)]
        for t in z_tiles:
            nc.any.memset(t[:], 0.0)
        h_tiles = [consts.tile([C, 2048], f8, name=f"ht{i}")
                   for i in range(6)]
        tgt = consts.tile([APC, 1024], f32)
        bigL = consts.tile([APC, 1024], f32)
        e_t = consts.tile([APC, 1024], f32)
        xts = consts.tile([APC, 1024], f32)
        lnout = consts.tile([APC, 1024], f32)
        acc_sp = consts.tile([APC, 1], f32)
        acc_xt = consts.tile([APC, 1], f32)

        # targets: one compare for all 32 anchors (int32 seg vs f32 class)
        nc.gpsimd.tensor_scalar(out=tgt[:], in0=mseg_sb[:],
                                scalar1=cls_sb[:, 0:1], scalar2=None,
                                op0=OP.is_equal)

        zps = zpp.tile([16, 512], f32)    # stage-A out, one chunk (1 bank)
        # logits psum (2 banks): chunk0 cols 0-511, chunk1 512-1023 on
        # rows 0-31 (PE dst base must be 0 for the 105-part contraction;
        # e9 routes subgroup sg's anchors to rows 4sg+j, others += 0).
        # start=True zeroes a whole 2KB psum region, so the once-started
        # logit edges and the per-anchor-restarted conv1 side columns
        # each need their own bank.
        ltt = ltp.tile([C, 1024], f32)
        edt = edp.tile([32, 64], f32)     # logit edge cols (2 ecol x 32 y)
        side = sdp.tile([C, 128], f32)    # conv1 edge cols

        def emit_conv1(a, hf):
            """conv1 + relu for anchor a, output-channel half hf."""
            ck, ci_in = chunk_of[a]
            xt = x_tiles[ck]
            abase = ci_in * XBYTES
            ht = h_tiles[a % 6]

            if True:
                for ci in range(2):
                    coff = 512 * ci
                    ct = c1p.tile([C, 512], f32, tag="c1",
                                  name=f"c1_{a}_{hf}_{ci}")
                    for p, (tA, tB) in enumerate(C1PAIRS):
                        dyA, dxA = tA // 3, tA % 3
                        lhs = view(w1_sb, p * 512 + hf * 256,
                                   [[128, 2], [1, 128]])
                        offA = abase + 31 + coff + 32 * dyA + dxA
                        rhs = view(xt, offA, [[64, 2], [1, 512]])
                        nc.tensor.matmul(ct[:], lhs, rhs,
                                         start=(p == 0), stop=(p == 5),
                                         perf_mode=PM.DoubleRow)
                    # bias + relu -> dense fp8 h (edge cols fixed below)
                    hview = ht[:, hf * 1024 + coff:hf * 1024 + coff + 512]
                    if hf == 0:
                        nc.scalar.activation(hview, ct[:], AF.Relu,
                                             bias=b1_sb[:, hf:hf + 1],
                                             scale=1.0)
                    else:
                        nc.vector.tensor_scalar(
                            out=hview, in0=ct[:],
                            scalar1=b1_sb[:, hf:hf + 1], scalar2=0.0,
                            op0=OP.add, op1=OP.max)
                # clean edge columns into the side region (in-bounds tap
                # pairs only): col 0 uses dx>=1 taps (pairs 1,2 + singles
                # 4,5), col 31 uses dx<=1 taps (pairs 0,1 + singles 3,4)
                for ei, (ocol, plist) in enumerate(
                        [(0, (1, 2, 4, 5)), (31, (0, 1, 3, 4))]):
                    sbase = hf * 64 + ei * 32
                    for pi, p in enumerate(plist):
                        tA = C1PAIRS[p][0]
                        dyA, dxA = tA // 3, tA % 3
                        roff = (abase + 64 + 32 * (dyA - 1) + dxA - 1
                                + ocol)
                        lhs = view(w1_sb, p * 512 + hf * 256,
                                   [[128, 2], [1, 128]])
                        rhs = view(xt, roff, [[64, 2], [32, 32]])
                        nc.tensor.matmul(side[:, sbase:sbase + 32], lhs,
                                         rhs, start=(pi == 0),
                                         stop=(pi == 3),
                                         perf_mode=PM.DoubleRow,
                                         skip_group_check=True)
                # overwrite h edge cols with the clean values
                hcols = view(ht, hf * 1024, [[31, 2], [CROP, CROP]])
                sin = side[:, hf * 64:hf * 64 + 64]
                if hf == 0:
                    nc.scalar.activation(hcols, sin, AF.Relu,
                                         bias=b1_sb[:, hf:hf + 1], scale=1.0)
                else:
                    nc.vector.tensor_scalar(
                        out=hcols, in0=sin,
                        scalar1=b1_sb[:, hf:hf + 1], scalar2=0.0,
                        op0=OP.add, op1=OP.max)

        def emit_tail_a(a, ci):
            """stage A + z copy for anchor a, px chunk ci (pipelined one
            anchor late so relu has a full conv1 of slack; chunks ride
            after different hf halves so the 1-bank zps never stalls)."""
            sg, j = divmod(a, GRP)
            ht = h_tiles[a % 6]
            z_sb = z_tiles[sg % 3]
            coff = 512 * ci
            # stage A: Z[9, f] over dense h (DR over hf halves, stride 1024)
            rhs = view(ht, coff, [[1024, 2], [1, 512]])
            lhs = view(w2_sb, 0, [[16, 2], [1, 9]])
            nc.tensor.matmul(zps[0:9, 0:512], lhs, rhs,
                             start=True, stop=True,
                             perf_mode=PM.DoubleRow)

            # Z copy: *1/2048, +b2 on center-tap row
            # (gpsimd cannot read PSUM, so alternate ACT / DVE)
            zout = view(z_sb, XD + 32 * j * XLEN + coff, [[1, 512]], 9)
            if (2 * a + ci) % 2 == 0:
                nc.scalar.activation(zout, zps[0:9, 0:512], AF.Identity,
                                     bias=b2_sb[32 * j:32 * j + 9, 0:1],
                                     scale=ZSC)
            else:
                nc.vector.tensor_scalar(
                    out=zout, in0=zps[0:9, 0:512], scalar1=ZSC,
                    scalar2=b2_sb[32 * j:32 * j + 9, 0:1],
                    op0=OP.mult, op1=OP.add)

        def emit_back(sg, nt):
            """stage B chunk nt (512 outs) for subgroup sg -> logits psum."""
            z_sb = z_tiles[sg % 3]
            coff = 512 * nt
            out = ltt[0:32, coff:coff + 512]
            for p, (tA, tB) in enumerate(ZPAIRS):
                dyA, dxA = tA // 3, tA % 3
                offA = XD - 33 + coff + 32 * dyA + dxA
                lhs = view(e9_sb, sg * 384 + p * 64, [[32, 2], [1, 32]], 105)
                rhs = view(z_sb, offA, [[64, 2], [1, 512]], 105)
                nc.tensor.matmul(out, lhs, rhs,
                                 start=(sg == 0 and p == 0), stop=False,
                                 perf_mode=PM.DoubleRow,
                                 skip_group_check=True)
            if nt == 0:
                # clean logits edge cols into ltt rows 64-95 (2 ecol x 32 y)
                for ei, (ocol, plist) in enumerate(
                        [(0, (1, 2, 4, 5)), (31, (0, 1, 3, 4))]):
                    for pi, p in enumerate(plist):
                        tA = ZPAIRS[p][0]
                        dyA, dxA = tA // 3, tA % 3
                        roff = XD + 32 * (dyA - 1) + dxA - 1 + ocol
                        lhs = view(e9_sb, sg * 384 + p * 64,
                                   [[32, 2], [1, 32]], 105)
                        rhs = view(z_sb, roff, [[64, 2], [32, 32]], 105)
                        nc.tensor.matmul(
                            edt[0:32, 32 * ei:32 * ei + 32],
                            lhs, rhs,
                            start=(sg == 0 and pi == 0), stop=False,
                            perf_mode=PM.DoubleRow,
                            skip_group_check=True)

        # interleaved emission, software-pipelined by one anchor:
        # PE stream: conv1(a,hf0), stageA(a-1,c0), [stage-B], conv1(a,hf1),
        # stageA(a-1,c1), ...
        pending = []
        for a in range(APC):
            emit_conv1(a, 0)
            if a >= 1:
                emit_tail_a(a - 1, 0)
            if pending:
                emit_back(*pending.pop(0))
            emit_conv1(a, 1)
            if a >= 1:
                emit_tail_a(a - 1, 1)
                if (a - 1) % GRP == GRP - 1:
                    sg = (a - 1) // GRP
                    pending += [(sg, 0), (sg, 1)]
        emit_tail_a(APC - 1, 0)
        emit_tail_a(APC - 1, 1)
        pending += [(NSG - 1, 0), (NSG - 1, 1)]
        for args in pending:
            emit_back(*args)

        # ---- final BCE over all 32 anchors, read straight from psum ----
        # interior cols 1..30 from the logits psum, edge cols from edt
        lint = view(ltt, 1, [[32, 32], [1, 30]], 32)
        tint = view(tgt, 1, [[32, 32], [1, 30]])
        eint = view(e_t, 1, [[32, 32], [1, 30]])
        xint = view(xts, 1, [[32, 32], [1, 30]])
        eedge = view(e_t, 0, [[31, 2], [CROP, CROP]])
        tedge = view(tgt, 0, [[31, 2], [CROP, CROP]])
        xedge = view(xts, 0, [[31, 2], [CROP, CROP]])
        # edge ops first: they only need edt (ready before the last
        # stage-B chunk), so they overlap its matmuls
        nc.scalar.activation(eedge, edt[:], AF.Exp, bias=0.0, scale=1.0)
        nc.vector.tensor_tensor(out=xedge, in0=edt[:], in1=tedge,
                                op=OP.mult)
        nc.scalar.activation(eint, lint, AF.Exp, bias=0.0, scale=1.0)
        nc.vector.tensor_tensor(out=xint, in0=lint, in1=tint, op=OP.mult)
        nc.scalar.activation(lnout[:], e_t[:], AF.Ln, bias=1.0, scale=1.0,
                             accum_out=acc_sp[:, 0:1])
        nc.vector.reduce_sum(acc_xt[:, 0:1], xts[:],
                             axis=mybir.AxisListType.X)
        osb = consts.tile([APC, 1], f32)
        nc.vector.tensor_tensor(out=osb[:], in0=acc_sp[:], in1=acc_xt[:],
                                op=OP.subtract)
        nc.sync.dma_start(out=outp[:, 0:1], in_=osb[:])

    nc.compile()
    return nc


def _get_program():
    if "nc" not in _cache:
        _cache["nc"] = _build_program()
    return _cache["nc"]


def host_inputs(feature_map, seg, anchors, labels, base_classes, W1, b1, W2,
                b2):
    """Device-layout input maps for all cores (layout/marshalling only)."""
    import ml_dtypes
    f8 = ml_dtypes.float8_e4m3

    feature_map = np.ascontiguousarray(feature_map, dtype=np.float32)
    seg = np.ascontiguousarray(seg, dtype=np.int32)
    anchors = np.asarray(anchors, dtype=np.int32)
    labels = np.asarray(labels, dtype=np.int32)
    base_classes = np.asarray(base_classes, dtype=np.int32)
    W1 = np.asarray(W1, dtype=np.float32)
    b1 = np.asarray(b1, dtype=np.float32)
    W2 = np.asarray(W2, dtype=np.float32)
    b2 = np.asarray(b2, dtype=np.float32)

    feat8 = feature_map.astype(f8)
    y0 = anchors[:, 2].astype(np.int64)
    x0 = anchors[:, 0].astype(np.int64)

    # staged crops: pitch 32, rows -2..33 zero-padded
    xstage = np.zeros((NANCH, C, XBYTES), dtype=f8)
    xa = xstage[:, :, :ALEN].reshape(NANCH, C, 36, 32)
    for a in range(NANCH):
        xa[a, :, 2:34, :] = feat8[:, y0[a]:y0[a] + CROP, x0[a]:x0[a] + CROP]

    # seg downsample (nearest): idx = floor(i*1280/320) = 4i
    seg_ds = seg[::4, ::4]
    mseg = np.zeros((NANCH, 1024), dtype=np.int32)
    for a in range(NANCH):
        mseg[a] = seg_ds[y0[a]:y0[a] + CROP,
                         x0[a]:x0[a] + CROP].reshape(1024)

    # w1: [ci, pair(6) x half(2) x tile(2) x co(128)]
    w1h = np.zeros((C, 6, 2, 2, 128), dtype=np.float32)
    for p, (tA, tB) in enumerate(C1PAIRS):
        for hf in range(2):
            w1h[:, p, hf, 0, :] = (
                W1[hf * 128:(hf + 1) * 128, :, tA // 3, tA % 3].T * W1S)
            if tB is not None:
                w1h[:, p, hf, 1, :] = (
                    W1[hf * 128:(hf + 1) * 128, :, tB // 3, tB % 3].T * W1S)
    w1h = np.ascontiguousarray(w1h.reshape(C, 3072).astype(f8))

    # w2: [ci, tile(2 halves) x 16 (9 taps + pad)]
    w2h = np.zeros((C, 2, 16), dtype=np.float32)
    for hf in range(2):
        w2h[:, hf, 0:9] = W2[0, hf * C:(hf + 1) * C].reshape(C, 9) * W2S
    w2h = np.ascontiguousarray(w2h.reshape(C, 32).astype(f8))

    # e9 pairs: [part, sg(8) x pair(6) x tile(2) x 32 (anchor rows)]
    # subgroup sg's anchor j (z partitions 32j..) -> output row 4sg+j
    e9 = np.zeros((C, NSG, 6, 2, 32), dtype=np.float32)
    for sg in range(NSG):
        for p, (tA, tB) in enumerate(ZPAIRS):
            for j in range(GRP):
                e9[32 * j + tA, sg, p, 0, GRP * sg + j] = 1.0
                if tB is not None:
                    e9[32 * j + tB, sg, p, 1, GRP * sg + j] = 1.0
    e9 = np.ascontiguousarray(e9.reshape(C, 3072).astype(f8))

    b1h = np.ascontiguousarray(b1.reshape(2, C).T * W1S)
    b2h = np.zeros((C, 1), dtype=np.float32)
    for j in range(GRP):
        b2h[32 * j + 4, 0] = b2[0]

    tgt_cls = base_classes[labels].astype(np.float32)

    in_maps = []
    for c in range(NCORES):
        sl = slice(c * APC, (c + 1) * APC)
        xsl = np.ascontiguousarray(
            xstage[sl].transpose(1, 0, 2).reshape(C, APC * XBYTES))
        in_maps.append({
            "xs": xsl,
            "msegp": np.ascontiguousarray(mseg[sl]),
            "clsp": np.ascontiguousarray(tgt_cls[sl].reshape(APC, 1)),
            "w1t": w1h,
            "w2t": w2h,
            "e9t": e9,
            "b1t": b1h,
            "b2t": b2h,
        })
    return in_maps


def kernel(feature_map, seg, anchors, labels, base_classes, W1, b1, W2, b2):
    global last_exec_time_ns, last_results
    import os
    from concourse.bass_utils import run_bass_kernel_spmd

    in_maps = host_inputs(feature_map, seg, anchors, labels, base_classes,
                          W1, b1, W2, b2)
    nc = _get_program()

    trace = os.environ.get("BASS_KERNEL_TRACE", "0") == "1"
    try:
        rb = run_bass_kernel_spmd(nc, in_maps, list(range(NCORES)),
                                  trace=trace)
    except ModuleNotFoundError:
        rb = run_bass_kernel_spmd(nc, in_maps, list(range(NCORES)),
                                  trace=False)
    last_results = rb
    last_exec_time_ns = rb.exec_time_ns

    partials = [float(rb.results[c]["out"][:, 0].sum(dtype=np.float64))
                for c in range(NCORES)]
    total = sum(partials) / CROP / CROP / (NANCH + 1e-10)
    return np.float32(total)
